# revision 4
# baseline (speedup 1.0000x reference)
"""Mixture-of-Experts Trainium2 kernel (8-core SPMD, token-sharded).

Reference computation (see problem): coarse top-K expert selection from the
gate applied to the global token sum, then dense K-expert FFN over all
tokens with per-token softmax gating over the K selected experts.

Strategy:
  * Host: coarse routing (gate @ x.sum over tokens, top-K) — O(E*D) flops,
    then gather the K selected experts' params, pre-cast to bf16 and
    pre-transpose x to feature-major [D, T]. Token-shard across 8 cores
    (T/8 tokens each), replicate the selected-expert weights.
  * Device (per core, all in bf16 matmuls with fp32 PSUM accumulation):
      gating logits [T,K] = xT.T @ gwsT (token-major), softmax over K
      per expert k:
        L1: h[F,T] = W1_k.T @ xT   (W1 chunks stationary, xT moving)
            gelu(tanh approx)+b1 on ACT, PSUM -> SBUF bf16
        L2: eo[T,D] = h_chunk.T @ W2_k  (h chunks stationary, W2 moving)
            combine: acc[T,D] (+)= eo * gw[:,k]  (one DVE scalar_tensor_tensor)
      out DMA [T, D] fp32.
"""

import os
import numpy as np
import ml_dtypes
from contextlib import ExitStack

import bass_rust as _bass_rust
import concourse.bass as bass
import concourse.mybir as mybir
import concourse.tile as tile
from concourse.vector_clock import ScopedClock
from concourse.bass_utils import run_bass_kernel_spmd

BF16 = mybir.dt.bfloat16
F32 = mybir.dt.float32
N_CORES = 8
P = 128


# ---------------------------------------------------------------------------
# Workaround for walrus "Too many sync wait commands": this walrus build
# accepts at most one semaphore wait in a single instruction's sync_info,
# but Tile's scheduler (and its kernel-tail drain) can attach several.
# Post-pass: move excess waits onto standalone EventSemaphore instructions
# inserted immediately before the offender on the same engine.
# ---------------------------------------------------------------------------
_split_ctr = [0]


def _split_multi_waits(nc):
    for f in nc.m.functions:
        for blk in f.blocks:
            insts = blk.instructions
            i = 0
            while i < len(insts):
                inst = insts[i]
                si = getattr(inst, "sync_info", None)
                waits = list(si.on_wait) if si is not None and si.on_wait else []
                if len(waits) > 1:
                    si.on_wait = waits[-1:]
                    for w in waits[:-1]:
                        _split_ctr[0] += 1
                        ev = mybir.InstEventSemaphore(
                            name=f"I-wsplit-{_split_ctr[0]}", ins=[], outs=[]
                        )
                        ev.engine = inst.engine
                        ev.sync_info = _bass_rust.SyncInfo(
                            on_wait=[w], on_update=[]
                        )
                        insts.insert(i, ev)
                        i += 1
                i += 1


# ---------------------------------------------------------------------------
# Device kernel
# ---------------------------------------------------------------------------
def build_moe_kernel(K: int, T: int, D: int, DF: int):
    """Per-core kernel: T tokens, D model dim, DF ffn dim, K selected experts."""
    assert T % 512 == 0 and D % P == 0 and DF % P == 0
    TT = T // P       # 128-token tiles
    TC = T // 512     # 512-token chunks
    DC = D // P       # D chunks of 128
    FC = DF // P      # F chunks of 128

    nc = bass.Bass("TRN2", target_bir_lowering=False)

    xT = nc.declare_dram_parameter("xT", [D, T], BF16, isOutput=False)
    w1s = nc.declare_dram_parameter("w1s", [K, D, DF], BF16, isOutput=False)
    w2s = nc.declare_dram_parameter("w2s", [K, DF, D], BF16, isOutput=False)
    gwsT = nc.declare_dram_parameter("gwsT", [P, DC * K], BF16, isOutput=False)
    gbb = nc.declare_dram_parameter("gbb", [P, K], F32, isOutput=False)
    b1p = nc.declare_dram_parameter("b1p", [K, P, FC], F32, isOutput=False)
    out = nc.declare_dram_parameter("out", [T, D], F32, isOutput=True)

    mult = mybir.AluOpType.mult
    add = mybir.AluOpType.add
    gelu_fn = mybir.ActivationFunctionType.Gelu_apprx_tanh
    exp_fn = mybir.ActivationFunctionType.Exp

    with tile.TileContext(nc) as tc:
        with ExitStack() as ctx:
            persist = ctx.enter_context(tc.tile_pool(name="persist", bufs=1))
            w1p = ctx.enter_context(tc.tile_pool(name="w1p", bufs=2 * DC))
            w2p = ctx.enter_context(tc.tile_pool(name="w2p", bufs=2 * FC))
            hp = ctx.enter_context(tc.tile_pool(name="hp", bufs=FC + 2))
            sm = ctx.enter_context(tc.tile_pool(name="sm", bufs=4))
            psA = ctx.enter_context(tc.tile_pool(name="psA", bufs=3, space="PSUM"))
            psB = ctx.enter_context(tc.tile_pool(name="psB", bufs=5, space="PSUM"))

            # ---- persistent loads ----
            xt = []
            for dc in range(DC):
                t = persist.tile([P, T], BF16, tag=f"xt{dc}", name=f"xt{dc}")
                nc.sync.dma_start(t[:], xT[dc * P:(dc + 1) * P, :])
                xt.append(t)
            gws_sb = persist.tile([P, DC * K], BF16, tag="gws", name="gws_sb")
            nc.sync.dma_start(gws_sb[:], gwsT[:])
            gbb_sb = persist.tile([P, K], F32, tag="gbb", name="gbb_sb")
            nc.sync.dma_start(gbb_sb[:], gbb[:])
            b1_sb = []
            for k in range(K):
                t = persist.tile([P, FC], F32, tag=f"b1_{k}", name=f"b1_{k}")
                nc.sync.dma_start(t[:], b1p[k])
                b1_sb.append(t)

            acc = [
                persist.tile([P, D], F32, tag=f"acc{t}", name=f"acc{t}")
                for t in range(TT)
            ]
            gw_sb = [
                persist.tile([P, K], F32, tag=f"gw{t}", name=f"gw{t}")
                for t in range(TT)
            ]

            # Prefetch expert 0 weights so their DMA is queued right after x
            # and overlaps the gating phase.
            w1t0 = []
            for dc in range(DC):
                t = w1p.tile([P, DF], BF16, tag="w1", name=f"w1_0_{dc}")
                nc.sync.dma_start(t[:], w1s[0, dc * P:(dc + 1) * P, :])
                w1t0.append(t)
            w2t0 = []
            for fc in range(FC):
                t = w2p.tile([P, D], BF16, tag="w2", name=f"w2_0_{fc}")
                nc.sync.dma_start(t[:], w2s[0, fc * P:(fc + 1) * P, :])
                w2t0.append(t)

            # ---- gating softmax (token-major) ----
            for tt in range(TT):
                pl = psB.tile([P, 512], F32, tag="po", name=f"pl{tt}")
                for dc in range(DC):
                    nc.tensor.matmul(
                        pl[:, 0:K],
                        xt[dc][:, tt * P:(tt + 1) * P],
                        gws_sb[:, dc * K:(dc + 1) * K],
                        start=(dc == 0),
                        stop=(dc == DC - 1),
                    )
                l_sb = sm.tile([P, K], F32, tag="l", name=f"l{tt}")
                nc.vector.tensor_add(l_sb[:], pl[:, 0:K], gbb_sb[:])
                negmax = sm.tile([P, 1], F32, tag="negmax", name=f"negmax{tt}")
                nc.vector.reduce_max(
                    negmax[:], l_sb[:], axis=mybir.AxisListType.X, negate=True
                )
                z = sm.tile([P, 1], F32, tag="z", name=f"z{tt}")
                nc.scalar.activation(
                    gw_sb[tt][:], l_sb[:], exp_fn,
                    bias=negmax[:, 0:1], accum_out=z[:, 0:1],
                )
                rz = sm.tile([P, 1], F32, tag="rz", name=f"rz{tt}")
                nc.vector.reciprocal(rz[:], z[:, 0:1])
                nc.vector.tensor_scalar_mul(gw_sb[tt][:], gw_sb[tt][:], rz[:, 0:1])

            # ---- experts ----
            for k in range(K):

                if k == 0:
                    w1t, w2t = w1t0, w2t0
                else:
                    w1t = []
                    for dc in range(DC):
                        t = w1p.tile([P, DF], BF16, tag="w1", name=f"w1_{k}_{dc}")
                        nc.sync.dma_start(t[:], w1s[k, dc * P:(dc + 1) * P, :])
                        w1t.append(t)
                    w2t = []
                    for fc in range(FC):
                        t = w2p.tile([P, D], BF16, tag="w2", name=f"w2_{k}_{fc}")
                        nc.sync.dma_start(t[:], w2s[k, fc * P:(fc + 1) * P, :])
                        w2t.append(t)

                # L1: h[F,T] = gelu(W1.T @ x + b1), feature-major
                ht = []
                for fc in range(FC):
                    h = hp.tile([P, T], BF16, tag="h", name=f"h_{k}_{fc}")
                    for tcc in range(TC):
                        ph = psA.tile([P, 512], F32, tag="ph", name=f"ph_{k}_{fc}_{tcc}")
                        for dc in range(DC):
                            nc.tensor.matmul(
                                ph[:],
                                w1t[dc][:, fc * P:(fc + 1) * P],
                                xt[dc][:, tcc * 512:(tcc + 1) * 512],
                                start=(dc == 0),
                                stop=(dc == DC - 1),
                            )
                        nc.scalar.activation(
                            h[:, tcc * 512:(tcc + 1) * 512], ph[:], gelu_fn,
                            bias=b1_sb[k][:, fc:fc + 1],
                        )
                    ht.append(h)

                # L2: eo[T,D] = h.T @ W2 ; acc (+)= eo * gw[:,k]
                for tt in range(TT):
                    po = psB.tile([P, 512], F32, tag="po", name=f"po_{k}_{tt}")
                    for fc in range(FC):
                        nc.tensor.matmul(
                            po[:, 0:D],
                            ht[fc][:, tt * P:(tt + 1) * P],
                            w2t[fc][:],
                            start=(fc == 0),
                            stop=(fc == FC - 1),
                        )
                    if k == 0:
                        nc.vector.tensor_scalar_mul(
                            acc[tt][:], po[:, 0:D], gw_sb[tt][:, 0:1]
                        )
                    else:
                        nc.vector.scalar_tensor_tensor(
                            acc[tt][:], po[:, 0:D], gw_sb[tt][:, k:k + 1],
                            acc[tt][:], op0=mult, op1=add,
                        )
                    if k == K - 1:
                        # store immediately: overlaps the remaining tiles'
                        # L2 matmuls instead of draining in a tail
                        nc.sync.dma_start(
                            out[tt * P:(tt + 1) * P, :], acc[tt][:]
                        )

    _split_multi_waits(nc)
    return nc


# ---------------------------------------------------------------------------
# Host wrapper
# ---------------------------------------------------------------------------
_NC_CACHE: dict = {}


def _get_nc(K: int, T: int, D: int, DF: int):
    key = (K, T, D, DF)
    if key not in _NC_CACHE:
        _NC_CACHE[key] = build_moe_kernel(K, T, D, DF)
    return _NC_CACHE[key]


def _softmax(x, axis=-1):
    m = np.max(x, axis=axis, keepdims=True)
    e = np.exp(x - m)
    return e / np.sum(e, axis=axis, keepdims=True)


def run(inputs: dict, trace: bool = False, tmpdir: str | None = None):
    x = np.asarray(inputs["x"], dtype=np.float32)
    gate_w = np.asarray(inputs["gate_w"], dtype=np.float32)
    gate_b = np.asarray(inputs["gate_b"], dtype=np.float32)
    w1 = np.asarray(inputs["w1"], dtype=np.float32)
    b1 = np.asarray(inputs["b1"], dtype=np.float32)
    w2 = np.asarray(inputs["w2"], dtype=np.float32)
    b2 = np.asarray(inputs["b2"], dtype=np.float32)
    K = int(inputs["num_available"])

    B, S, D = x.shape
    DF = w1.shape[2]
    Ttot = B * S
    T = Ttot // N_CORES
    DC = D // P
    FC = DF // P

    # Coarse routing on host (tiny): gate applied to the global token sum.
    ksum = x.sum(axis=(0, 1))
    coarse = gate_w @ ksum + gate_b
    idx = np.argsort(-coarse, kind="stable")[:K]

    gws = gate_w[idx]                      # [K, D]
    gbs = gate_b[idx]                      # [K]
    w1s = np.ascontiguousarray(w1[idx]).astype(ml_dtypes.bfloat16)   # [K,D,DF]
    b1s = np.ascontiguousarray(b1[idx], dtype=np.float32)            # [K,DF]
    w2s = np.ascontiguousarray(w2[idx]).astype(ml_dtypes.bfloat16)   # [K,DF,D]
    b2s = np.ascontiguousarray(b2[idx], dtype=np.float32)            # [K,D]

    # gwsT packed [128, DC*K]: column dc*K+k = gate row k, D slice dc
    gwsT = np.ascontiguousarray(
        gws.T.reshape(DC, P, K).transpose(1, 0, 2).reshape(P, DC * K)
    ).astype(ml_dtypes.bfloat16)
    gbb = np.ascontiguousarray(np.broadcast_to(gbs[None, :], (P, K)), dtype=np.float32)
    b1p = np.ascontiguousarray(b1s.reshape(K, FC, P).transpose(0, 2, 1), dtype=np.float32)

    xf = x.reshape(Ttot, D)
    xT_bf = np.ascontiguousarray(xf.T).astype(ml_dtypes.bfloat16)    # [D, Ttot]

    nc = _get_nc(K, T, D, DF)
    in_maps = []
    for c in range(N_CORES):
        in_maps.append({
            "xT": np.ascontiguousarray(xT_bf[:, c * T:(c + 1) * T]),
            "w1s": w1s,
            "w2s": w2s,
            "gwsT": gwsT,
            "gbb": gbb,
            "b1p": b1p,
        })

    res = run_bass_kernel_spmd(
        nc, in_maps, list(range(N_CORES)), trace=trace, tmpdir=tmpdir
    )
    outp = np.concatenate(
        [res.results[c]["out"] for c in range(N_CORES)], axis=0
    ).reshape(B, S, D).astype(np.float32)

    # b2 contribution (zero in this problem's inputs; exact host-side fallback)
    if np.any(b2s):
        logits = xf @ gws.T + gbs[None, :]
        gwh = _softmax(logits, axis=1)
        outp = outp + (gwh @ b2s).reshape(B, S, D)

    return outp, res


def kernel(**inputs) -> np.ndarray:
    outp, _ = run(inputs, trace=False)
    return outp


